# revision 3
# baseline (speedup 1.0000x reference)
"""FLC pooling (FFT2 -> center-crop low freqs -> IFFT2, real part) on 8 trn2 cores.

Math: per (n,c) slice, out = Re(M @ X @ M.T) where M (112x224) is the
1D fft->fftshift->crop->ifftshift->ifft operator. Im(M) is exactly rank-1
(= outer(a, b) with a[u] = c*(-1)^u), so with R = Re(M):

    out = R @ X @ R.T - (b.T @ X @ b) * a0sq * checkerboard

Device pipeline per slice (all fp32, matmuls as float32r at full PE rate):
    G = [R; b] (113x224)
    W1  = G @ X                  (pass 1, PE; rhs = X batched 2 slices, N=448)
    W1T = transpose(W1)          (PE transpose via identity, 2x 112-col chunks)
    V   = G @ W1T = (G X G.T)^T  (pass 2, PE; rhs batched 4 slices, N=452)
    s   = b.T X b                (tiny matmul broadcasting s to all partitions)
    outT_slice = V[:112,:112] + s * (-a0sq * checkerboard)   (one fused DVE op)
Host: out = outT_slice.T (free re-layout during unshard).

Sharding: batch*channel = 1024 independent slices -> 128 per core.
"""

import sys

sys.path.insert(0, "/opt/trn_rl_repo")

import numpy as np

import concourse.bass as bass  # noqa: F401
import concourse.mybir as mybir
import concourse.tile as tile
from concourse import bacc
from concourse.bass_utils import run_bass_kernel_spmd

N = 224
NH = 112
NG = 113  # rows of G = [R; b]
B, C = 16, 64
NCORES = 8
NSLICES = B * C // NCORES  # 128 slices per core
F32 = mybir.dt.float32
F32R = mybir.dt.float32r


def _build_consts():
    F = np.fft.fft(np.eye(N), axis=0, norm="forward")
    M = np.fft.ifft(
        np.fft.ifftshift(np.fft.fftshift(F, axes=0)[N // 4 : 3 * N // 4], axes=0),
        axis=0,
        norm="forward",
    )
    R, S = M.real, M.imag
    u, sv, vt = np.linalg.svd(S)
    a = u[:, 0] * np.sqrt(sv[0])
    b = vt[0] * np.sqrt(sv[0])
    if np.abs(S - np.outer(a, b)).max() > 1e-10:
        a, b = -a, -b
    assert np.abs(S - np.outer(a, b)).max() < 1e-12
    G = np.vstack([R, b[None, :]])  # [113, 224]
    # gt[c][i, u] = G[u, 112c + i]  (G^T row chunks; lhsT for both passes)
    gt = np.ascontiguousarray(G.T.reshape(2, NH, NG)).astype(np.float32)
    # bbc[c][j, m] = b[112c + j] for all m (column-broadcast b; lhsT for s)
    bbc = np.ascontiguousarray(
        np.repeat(b.reshape(2, NH, 1), NH, axis=2)
    ).astype(np.float32)
    a0sq = float(a[0] * a[0])  # = 1/224
    vv = np.arange(NH)
    cneg = (-a0sq * ((-1.0) ** (vv[:, None] + vv[None, :]))).astype(np.float32)
    ident = np.eye(NG, dtype=np.float32)
    return gt, bbc, cneg, ident


def _build_nc():
    nc = bacc.Bacc("TRN2", target_bir_lowering=False, debug=False)
    x = nc.dram_tensor("x", [NSLICES, N, N], F32R, kind="ExternalInput").ap()
    gt = nc.dram_tensor("gt", [2, NH, NG], F32R, kind="ExternalInput").ap()
    bbc = nc.dram_tensor("bbc", [2, NH, NH], F32R, kind="ExternalInput").ap()
    cneg = nc.dram_tensor("cneg", [NH, NH], F32, kind="ExternalInput").ap()
    ident = nc.dram_tensor("ident", [NG, NG], F32, kind="ExternalInput").ap()
    outT = nc.dram_tensor("outT", [NSLICES, NH, NH], F32, kind="ExternalOutput").ap()

    mult = mybir.AluOpType.mult
    add = mybir.AluOpType.add

    with tile.TileContext(nc) as tc:
        with (
            tc.tile_pool(name="consts", bufs=1) as cpool,
            tc.tile_pool(name="xt", bufs=8) as xpool,
            tc.tile_pool(name="w1sb", bufs=4) as w1sb_pool,
            tc.tile_pool(name="w1t4", bufs=2) as w1t4_pool,
            tc.tile_pool(name="vout", bufs=3) as vout_pool,
            tc.tile_pool(name="w1p", bufs=2, space="PSUM") as w1psum,
            tc.tile_pool(name="w1tp", bufs=2, space="PSUM") as w1tpsum,
            tc.tile_pool(name="v4p", bufs=2, space="PSUM") as vpsum,
            tc.tile_pool(name="s4p", bufs=2, space="PSUM") as spsum,
        ):
            gt_sb = cpool.tile([NH, 2, NG], F32R)
            nc.sync.dma_start(gt_sb[:], gt.rearrange("c i u -> i c u"))
            bbc_sb = cpool.tile([NH, 2, NH], F32R)
            nc.sync.dma_start(bbc_sb[:], bbc.rearrange("c j m -> j c m"))
            cneg_sb = cpool.tile([NH, NH], F32)
            nc.sync.dma_start(cneg_sb[:], cneg)
            id_sb = cpool.tile([NG, NG], F32)
            nc.sync.dma_start(id_sb[:], ident)

            for g in range(NSLICES // 4):
                # [part j%112, chunk h=j//112, slice-in-group, u] for pass 2
                w1t4 = w1t4_pool.tile([NH, 2, 4, NG], F32R)
                for p in range(2):
                    s0 = 4 * g + 2 * p
                    # [part i%112, chunk c=i//112, slice-in-pair, j]
                    xt = xpool.tile([NH, 2, 2, N], F32R, tag="xt")
                    for c in range(2):
                        nc.sync.dma_start(
                            xt[:, c],
                            x[s0 : s0 + 2, c * NH : (c + 1) * NH, :].rearrange(
                                "s p j -> p s j"
                            ),
                        )
                    w1p = w1psum.tile([NG, 2, N], F32)  # W1 for the pair
                    for c in range(2):
                        nc.tensor.matmul(
                            w1p[:],
                            gt_sb[:, c, :],
                            xt[:, c],
                            start=(c == 0),
                            stop=(c == 1),
                        )
                    w1sb = w1sb_pool.tile([NG, 2, N], F32)
                    nc.vector.tensor_copy(w1sb[:], w1p[:])
                    for si in range(2):
                        sl = 2 * p + si
                        w1tp = w1tpsum.tile([NH, 2, NG], F32)
                        for h in range(2):
                            nc.tensor.transpose(
                                w1tp[:, h, :],
                                w1sb[:, si, h * NH : (h + 1) * NH],
                                id_sb[:],
                            )
                        nc.scalar.copy(w1t4[:, :, sl, :], w1tp[:])
                v4 = vpsum.tile([NG, 4, NG], F32)
                s4 = spsum.tile([NH, 4], F32)
                for h in range(2):
                    nc.tensor.matmul(
                        v4[:],
                        gt_sb[:, h, :],
                        w1t4[:, h],
                        start=(h == 0),
                        stop=(h == 1),
                    )
                for h in range(2):
                    nc.tensor.matmul(
                        s4[:],
                        bbc_sb[:, h, :],
                        w1t4[:, h, :, NH : NH + 1],
                        start=(h == 0),
                        stop=(h == 1),
                    )
                vout = vout_pool.tile([NH, 4, NH], F32)
                for sl in range(4):
                    # vout = cneg * s + V  (fused correction + PSUM eviction)
                    nc.vector.scalar_tensor_tensor(
                        out=vout[:, sl, :],
                        in0=cneg_sb[:],
                        scalar=s4[:, sl : sl + 1],
                        in1=v4[0:NH, sl, 0:NH],
                        op0=mult,
                        op1=add,
                    )
                nc.sync.dma_start(
                    outT[4 * g : 4 * g + 4].rearrange("s v u -> v s u"), vout[:]
                )
    nc.compile()
    return nc


_CACHE: dict = {}


def _get_compiled():
    if "nc" not in _CACHE:
        _CACHE["consts"] = _build_consts()
        _CACHE["nc"] = _build_nc()
    return _CACHE["nc"], _CACHE["consts"]


def run(x: np.ndarray, trace: bool = False):
    """Returns (out [16,64,112,112] fp32, BassKernelResults)."""
    nc, (gt, bbc, cneg, ident) = _get_compiled()
    x = np.ascontiguousarray(np.asarray(x, dtype=np.float32))
    shards = x.reshape(NCORES, NSLICES, N, N)
    in_maps = [
        {"x": shards[i], "gt": gt, "bbc": bbc, "cneg": cneg, "ident": ident}
        for i in range(NCORES)
    ]
    last_err = None
    for _attempt in range(3):
        try:
            res = run_bass_kernel_spmd(
                nc, in_maps, core_ids=list(range(NCORES)), trace=trace
            )
            break
        except Exception as e:  # transient NRT device errors: retry
            last_err = e
    else:
        raise last_err
    outT = np.stack([r["outT"] for r in res.results], axis=0)
    out = np.ascontiguousarray(
        outT.reshape(B * C, NH, NH).transpose(0, 2, 1)
    ).reshape(B, C, NH, NH)
    return out, res


def kernel(x: np.ndarray) -> np.ndarray:
    out, _ = run(x, trace=False)
    return out


# revision 5
# speedup vs baseline: 1.5523x; 1.5523x over previous
"""FLC pooling (FFT2 -> center-crop low freqs -> IFFT2, real part) on 8 trn2 cores.

Math: per (n,c) slice, out = Re(M @ X @ M.T) where M (112x224) is the 1D
fft -> fftshift -> crop -> ifftshift -> ifft operator. Im(M) is exactly
rank-1 (= outer(a, b), a[u] = a0*(-1)^u), so with R = Re(M), G = [R; b]:

    out_ext = G @ X @ G.T            (113x113; [112,112] entry = b'Xb)
    out = out_ext[:112,:112] - out_ext[112,112] * a0^2 * checkerboard

Device pipeline (fp16 operands, fp32 PSUM accumulation):
    W1T = X.T @ G.T      pass 1: stationary = X chunks (fp16), streams G.T;
                         produces the *transposed* intermediate directly,
                         so no PE transposes / identity are needed.
    V   = G @ W1T        pass 2: = out_ext^T, 4 slices batched (N=452)
    s   = b.T X b        tiny matmul against W1T col 112, broadcast to
                         all partitions via a constant-column lhsT
    vout = cneg*s + V    one fused DVE scalar_tensor_tensor per slice
Host unshard transposes each 112x112 slice (free re-layout).

x is loaded by gpsimd casting DMA (fp32 HBM -> fp16 SBUF), keeping the
Sync engine free and halving SBUF traffic; 4 slices per DMA descriptor.

Sharding: batch*channel = 1024 independent (n,c) slices -> 128 per core.
"""

import sys

sys.path.insert(0, "/opt/trn_rl_repo")

import numpy as np

import concourse.bass as bass  # noqa: F401
import concourse.mybir as mybir
import concourse.tile as tile
from concourse import bacc
from concourse.bass_utils import run_bass_kernel_spmd

N = 224
NH = 112
NG = 113  # rows of G = [R; b]
B, C = 16, 64
NCORES = 8
NSLICES = B * C // NCORES  # 128 slices per core
F32 = mybir.dt.float32
F16 = mybir.dt.float16


def _build_consts():
    F = np.fft.fft(np.eye(N), axis=0, norm="forward")
    M = np.fft.ifft(
        np.fft.ifftshift(np.fft.fftshift(F, axes=0)[N // 4 : 3 * N // 4], axes=0),
        axis=0,
        norm="forward",
    )
    R, S = M.real, M.imag
    u, sv, vt = np.linalg.svd(S)
    a = u[:, 0] * np.sqrt(sv[0])
    b = vt[0] * np.sqrt(sv[0])
    if np.abs(S - np.outer(a, b)).max() > 1e-10:
        a, b = -a, -b
    assert np.abs(S - np.outer(a, b)).max() < 1e-12
    G = np.vstack([R, b[None, :]])  # [113, 224]
    # gt16[c][i, u] = G[u, 112c + i]  (G^T row chunks, fp16)
    gt16 = np.ascontiguousarray(G.T.reshape(2, NH, NG)).astype(np.float16)
    # bbc16[c][j, m] = b[112c + j] for all m (column-broadcast b)
    bbc16 = np.ascontiguousarray(
        np.repeat(b.reshape(2, NH, 1), NH, axis=2)
    ).astype(np.float16)
    a0sq = float(a[0] * a[0])  # = 1/224
    vv = np.arange(NH)
    cneg = (-a0sq * ((-1.0) ** (vv[:, None] + vv[None, :]))).astype(np.float32)
    return gt16, bbc16, cneg


def _build_nc():
    nc = bacc.Bacc("TRN2", target_bir_lowering=False, debug=False)
    x = nc.dram_tensor("x", [NSLICES, N, N], F32, kind="ExternalInput").ap()
    gt = nc.dram_tensor("gt", [2, NH, NG], F16, kind="ExternalInput").ap()
    bbc = nc.dram_tensor("bbc", [2, NH, NH], F16, kind="ExternalInput").ap()
    cneg = nc.dram_tensor("cneg", [NH, NH], F32, kind="ExternalInput").ap()
    outT = nc.dram_tensor("outT", [NSLICES, NH, NH], F32, kind="ExternalOutput").ap()

    mult = mybir.AluOpType.mult
    add = mybir.AluOpType.add

    with tile.TileContext(nc) as tc:
        with (
            tc.tile_pool(name="consts", bufs=1) as cpool,
            tc.tile_pool(name="xt", bufs=3) as xpool,
            tc.tile_pool(name="w1t4", bufs=2) as w1t4_pool,
            tc.tile_pool(name="vout", bufs=3) as vout_pool,
            tc.tile_pool(name="w1tp", bufs=4, space="PSUM") as w1tpsum,
            tc.tile_pool(name="v4p", bufs=2, space="PSUM") as vpsum,
            tc.tile_pool(name="s4p", bufs=2, space="PSUM") as spsum,
        ):
            gt_sb = cpool.tile([NH, 2, NG], F16)
            nc.sync.dma_start(gt_sb[:], gt.rearrange("c i u -> i c u"))
            bbc_sb = cpool.tile([NH, 2, NH], F16)
            nc.sync.dma_start(bbc_sb[:], bbc.rearrange("c j m -> j c m"))
            cneg_sb = cpool.tile([NH, NH], F32)
            nc.sync.dma_start(cneg_sb[:], cneg)

            for g in range(NSLICES // 4):
                # xt[p, c, s, j] = X_s[112c + p, j], fp16 (cast in DMA)
                xt = xpool.tile([NH, 2, 4, N], F16, tag="xt")
                for c in range(2):
                    nc.gpsimd.dma_start(
                        xt[:, c],
                        x[4 * g : 4 * g + 4, c * NH : (c + 1) * NH, :].rearrange(
                            "s p j -> p s j"
                        ),
                    )
                # w1t4[p, h, s, u] = W1T_s[112h + p, u] = W1_s[u, 112h + p]
                w1t4 = w1t4_pool.tile([NH, 2, 4, NG], F16)
                for sl in range(4):
                    w1tp = w1tpsum.tile([NH, 2, NG], F32)
                    for h in range(2):  # W1T row chunk (j)
                        for c in range(2):  # contraction chunk (i)
                            nc.tensor.matmul(
                                w1tp[:, h, :],
                                xt[:, c, sl, h * NH : (h + 1) * NH],
                                gt_sb[:, c, :],
                                start=(c == 0),
                                stop=(c == 1),
                            )
                    nc.scalar.copy(w1t4[:, :, sl, :], w1tp[:])
                v4 = vpsum.tile([NG, 4, NG], F32)
                s4 = spsum.tile([NH, 4], F32)
                for h in range(2):
                    nc.tensor.matmul(
                        v4[:],
                        gt_sb[:, h, :],
                        w1t4[:, h],
                        start=(h == 0),
                        stop=(h == 1),
                    )
                for h in range(2):
                    nc.tensor.matmul(
                        s4[:],
                        bbc_sb[:, h, :],
                        w1t4[:, h, :, NH : NH + 1],
                        start=(h == 0),
                        stop=(h == 1),
                    )
                vout = vout_pool.tile([NH, 4, NH], F32)
                for sl in range(4):
                    # vout = cneg * s + V  (fused correction + PSUM eviction)
                    nc.vector.scalar_tensor_tensor(
                        out=vout[:, sl, :],
                        in0=cneg_sb[:],
                        scalar=s4[:, sl : sl + 1],
                        in1=v4[0:NH, sl, 0:NH],
                        op0=mult,
                        op1=add,
                    )
                nc.sync.dma_start(
                    outT[4 * g : 4 * g + 4].rearrange("s v u -> v s u"), vout[:]
                )
    nc.compile()
    return nc


_CACHE: dict = {}


def _get_compiled():
    if "nc" not in _CACHE:
        _CACHE["consts"] = _build_consts()
        _CACHE["nc"] = _build_nc()
    return _CACHE["nc"], _CACHE["consts"]


def run(x: np.ndarray, trace: bool = False):
    """Returns (out [16,64,112,112] fp32, BassKernelResults)."""
    nc, (gt16, bbc16, cneg) = _get_compiled()
    x = np.ascontiguousarray(np.asarray(x, dtype=np.float32))
    shards = x.reshape(NCORES, NSLICES, N, N)
    in_maps = [
        {"x": shards[i], "gt": gt16, "bbc": bbc16, "cneg": cneg}
        for i in range(NCORES)
    ]
    last_err = None
    for _attempt in range(3):
        try:
            res = run_bass_kernel_spmd(
                nc, in_maps, core_ids=list(range(NCORES)), trace=trace
            )
            break
        except Exception as e:  # transient NRT device errors: retry
            last_err = e
    else:
        raise last_err
    outT = np.stack([r["outT"] for r in res.results], axis=0)
    out = np.ascontiguousarray(
        outT.reshape(B * C, NH, NH).transpose(0, 2, 1)
    ).reshape(B, C, NH, NH)
    return out, res


def kernel(x: np.ndarray) -> np.ndarray:
    out, _ = run(x, trace=False)
    return out
